# revision 1
# baseline (speedup 1.0000x reference)
"""DiagonalAttention Trainium2 kernel (Bass/Tile), data-parallel over batch on 8 cores.

Reference computation (per batch b):
    r1 = relu(x1 @ W.T) * diagonal          [L, H]
    r2 = relu(x2 @ W.T)                     [L, H]
    s  = r1 @ r2.T + (1-mask)*NEG           [L, L]
    out = softmax(s, -1) @ x2               [L, D]

Device strategy per core (2 batches/core):
  - host: transpose x1/x2 to [D, L], tf32-round them + W (fp32r matmuls run at
    1 cyc/row on the PE), cast x2 to bf16 for the output matmul.
  - proj (fp32r): rT[h, l] accumulated over d-chunks, relu on ScalarE -> fp32r.
  - scores (fp32r): psum[i=128, j=2048], mask row added via a K=1 bf16 starter
    matmul; ScalarE copies scores to SBUF (single-reader psum), VectorE row-max,
    ScalarE exp(s-max) -> bf16 E with fused row-sum (accum_out).
  - E transposed 128x128 on the PE (bf16), bmm3 = ET.T @ x2_bf16 accumulated in
    psum, scaled by 1/z on VectorE during psum->SBUF copy, DMA out.

Every PE/ACT/DVE instruction is limited to ONE semaphore wait by walrus codegen;
tiny "absorber" ops (PE corner-transposes into a persistent psum tile, DVE corner
reads/memsets, ACT bias bounce) pre-observe semaphores so no instruction ever
needs two.
"""
import numpy as np

B, L, D, H = 16, 2048, 1024, 1024
NCORES = 8
B_LOC = B // NCORES
NEG = -10000.0

ND = D // 128   # d chunks
NH = H // 128   # h chunks
NI = L // 128   # i chunks per batch
SW = 256        # proj slab width (moving-dim of fp32r matmuls)
NS = L // SW    # slabs per batch
IPS = SW // 128  # i-chunks per slab
JW = 512        # bmm2 moving width
NJ = L // JW    # j chunks in bmm2
NJ3 = L // 128  # j chunks in bmm3 (stationary ET tiles)

_PROG = None


def tf32_round(x):
    xi = np.ascontiguousarray(x, dtype=np.float32).view(np.uint32)
    return ((xi + 0x1000) & 0xFFFFE000).view(np.float32)


def _build_program(use_mask_starter=True, separate_w1=False, sw=SW):
    import concourse.bass as bass
    import concourse.tile as tile
    from concourse import mybir
    from concourse.bass import _add_dep_helper
    from concourse.masks import make_identity

    def order(first, then):
        _add_dep_helper(then.ins, first.ins, sync=False, reason="order")
        return then

    dt = mybir.dt
    nc = bass.Bass("TRN2", target_bir_lowering=False, debug=False)

    x1T = nc.dram_tensor("x1T", [B_LOC, D, L], dt.float32r, kind="ExternalInput").ap()
    x2T = nc.dram_tensor("x2T", [B_LOC, D, L], dt.float32r, kind="ExternalInput").ap()
    WT = nc.dram_tensor("WT", [D, H], dt.float32r, kind="ExternalInput").ap()
    W1T = nc.dram_tensor("W1T", [D, H], dt.float32r, kind="ExternalInput").ap()
    x2n = nc.dram_tensor("x2n", [B_LOC, L, D], dt.bfloat16, kind="ExternalInput").ap()
    ns = L // sw
    ips = sw // 128
    mrow = nc.dram_tensor("mrow", [B_LOC, 1, L], dt.bfloat16, kind="ExternalInput").ap()
    out = nc.dram_tensor("out", [B_LOC, L, D], dt.float32, kind="ExternalOutput").ap()
    dscr_list = [nc.dram_tensor(f"dscr{k}", [1, 4], dt.float32r).ap()
                 for k in range(256)]
    scr_idx = [0]

    with tile.TileContext(nc) as tc:
        with (
            tc.tile_pool(name="const", bufs=1) as cpool,
            tc.tile_pool(name="big", bufs=1) as bigp,
            tc.tile_pool(name="slab", bufs=2) as slabp,
            tc.tile_pool(name="work", bufs=1) as workp,
            tc.tile_pool(name="work2", bufs=2) as work2p,
            tc.tile_pool(name="outp", bufs=4) as outp,
            tc.tile_pool(name="ps_s", bufs=1, space="PSUM") as ps_s,
            tc.tile_pool(name="ps_p", bufs=1, space="PSUM") as ps_p,
            tc.tile_pool(name="ps_sm", bufs=2, space="PSUM") as ps_sm,
            tc.tile_pool(name="ps_ab", bufs=1, space="PSUM") as ps_abp,
        ):
            identbf = cpool.tile([128, 128], dt.bfloat16, tag="identbf")
            make_identity(nc, identbf[:])
            identr = cpool.tile([32, 32], dt.float32r, tag="identr")
            make_identity(nc, identr[:])
            ones_bf = cpool.tile([1, 128], dt.bfloat16, tag="ones_bf")
            nc.vector.memset(ones_bf[:], 1.0)
            dve_ab = cpool.tile([1, 8], dt.float32, tag="dve_ab")
            sbscr = cpool.tile([1, 1024], dt.float32r, tag="sbscr")
            act_ab = cpool.tile([1, 8], dt.float32, tag="act_ab")
            echo = cpool.tile([32, 32], dt.float32r, tag="echo")

            ps_ab_t = ps_abp.tile([32, 64], dt.bfloat16, tag="ps_ab")
            ps_ab = ps_ab_t[:, 0:64].bitcast(dt.float32r)
            # warmup: PE observes GPSIMD (identity producer)
            nc.tensor.transpose(ps_ab, identr[:], identr[:])

            def absorb_r(corner_ap):
                # PE pre-observes the semaphore guarding corner_ap (1 wait)
                return nc.tensor.transpose(ps_ab, corner_ap, identr[:])

            def absorb_bf(corner_ap):
                return nc.tensor.transpose(ps_ab_t[:, 0:32], corner_ap,
                                           identbf[0:32, 0:32])

            def sp_sync(dep_inst):
                # SP observes dep_inst's engine tick via a write-once scratch
                d = nc.sync.dma_start(dscr_list[scr_idx[0]][0:1, 0:1],
                                      WT[0:1, 0:1])
                scr_idx[0] += 1
                _add_dep_helper(d.ins, dep_inst.ins, sync=True,
                                reason="sp sync absorb")
                return d

            def sp_absorb(corner_ap):
                # SP pre-observes the DMA semaphore guarding corner_ap (1 wait)
                v = corner_ap.bitcast(dt.float32r)
                n = v.free_size()
                nc.sync.dma_start(dscr_list[scr_idx[0]][0:1, 0:n], v)
                scr_idx[0] += 1

            # resident weights
            wt = bigp.tile([128, ND, H], dt.float32r, tag="wt")
            for dc in range(ND):
                nc.sync.dma_start(wt[:, dc, :],
                                  WT.rearrange("(c p) h -> p c h", p=128)[:, dc, :])
            for dc in range(ND):
                absorb_r(wt[0:32, dc, 0:32])
            if separate_w1:
                w1t = bigp.tile([128, ND, H], dt.float32r, tag="w1t")
                for dc in range(ND):
                    nc.sync.dma_start(
                        w1t[:, dc, :],
                        W1T.rearrange("(c p) h -> p c h", p=128)[:, dc, :])
                for dc in range(ND):
                    absorb_r(w1t[0:32, dc, 0:32])
            else:
                # diagonal == 1: W1 is W, share the resident tile (the W1T dram
                # input is still bound; a token DMA keeps it referenced)
                w1t = wt
                w1tok = cpool.tile([1, 8], dt.float32r, tag="w1tok")
                nc.sync.dma_start(w1tok[:], W1T[0:1, 0:8])

            # resident per-batch tensors
            r2T = bigp.tile([128, NH, L], dt.float32r, tag="r2T")
            t_x2n = bigp.tile([128, NJ3, D], dt.bfloat16, tag="t_x2n")
            t_m = workp.tile([1, L], dt.bfloat16, tag="t_m")
            sco = workp.tile([128, L], dt.float32, tag="sco")
            te = workp.tile([128, L], dt.bfloat16, tag="te")
            tET = workp.tile([128, NJ3, 128], dt.bfloat16, tag="tET")

            prev_relu_corner = [None]
            first_chunk = [True]
            slab_alloc_count = [0]
            slot_last_mm = [None, None]
            prev_out = [None]
            last_bmm3_mm = [None]
            last_starter = [None]
            for b in range(B_LOC):
                # ---- batch loads ----
                if last_bmm3_mm[0] is not None:
                    sp_sync(last_bmm3_mm[0])
                for jc in range(NJ3):
                    nc.sync.dma_start(
                        t_x2n[:, jc, :],
                        x2n[b].rearrange("(c p) d -> p c d", p=128)[:, jc, :])
                for jc in range(NJ3):
                    absorb_bf(t_x2n[0:32, jc, 0:32])
                if last_starter[0] is not None:
                    sp_sync(last_starter[0])
                nc.sync.dma_start(t_m[:], mrow[b])

                # ---- proj2: r2T = relu(W @ x2T) over all slabs ----
                for s in range(ns):
                    slot = slab_alloc_count[0] % 2
                    slab_alloc_count[0] += 1
                    if slot_last_mm[slot] is not None:
                        sp_sync(slot_last_mm[slot])
                    xs = slabp.tile([128, ND, sw], dt.float32r, tag="xslab")
                    for dc in range(ND):
                        nc.sync.dma_start(
                            xs[:, dc, :],
                            x2T[b].rearrange("(c p) l -> p c l", p=128)[
                                :, dc, s * sw:(s + 1) * sw],
                        )
                    for dc in range(ND):
                        absorb_r(xs[0:32, dc, 0:32])
                    for hc in range(NH):
                        if prev_relu_corner[0] is not None:
                            absorb_r(prev_relu_corner[0])
                        psp = ps_p.tile([128, sw], dt.float32, tag="psp")
                        for dc in range(ND):
                            mm = nc.tensor.matmul(
                                psp[:], wt[:, dc, hc * 128:(hc + 1) * 128],
                                xs[:, dc, :],
                                start=(dc == 0), stop=(dc == ND - 1),
                            )
                            slot_last_mm[slot] = mm
                        nc.scalar.activation(
                            r2T[:, hc, s * sw:(s + 1) * sw], psp[:],
                            mybir.ActivationFunctionType.Relu)
                        prev_relu_corner[0] = r2T[0:32, hc, s * sw:s * sw + 32]

                # ---- proj1 + attention, slab by slab ----
                for s in range(ns):
                    slot = slab_alloc_count[0] % 2
                    slab_alloc_count[0] += 1
                    if slot_last_mm[slot] is not None:
                        sp_sync(slot_last_mm[slot])
                    xs = slabp.tile([128, ND, sw], dt.float32r, tag="xslab")
                    for dc in range(ND):
                        nc.sync.dma_start(
                            xs[:, dc, :],
                            x1T[b].rearrange("(c p) l -> p c l", p=128)[
                                :, dc, s * sw:(s + 1) * sw],
                        )
                    for dc in range(ND):
                        absorb_r(xs[0:32, dc, 0:32])
                    r1s = slabp.tile([128, NH, sw], dt.float32r, tag="r1slab")
                    for hc in range(NH):
                        if prev_relu_corner[0] is not None:
                            absorb_r(prev_relu_corner[0])
                        psp = ps_p.tile([128, sw], dt.float32, tag="psp")
                        for dc in range(ND):
                            mm = nc.tensor.matmul(
                                psp[:], w1t[:, dc, hc * 128:(hc + 1) * 128],
                                xs[:, dc, :],
                                start=(dc == 0), stop=(dc == ND - 1),
                            )
                            slot_last_mm[slot] = mm
                        nc.scalar.activation(
                            r1s[:, hc, :], psp[:],
                            mybir.ActivationFunctionType.Relu)
                        prev_relu_corner[0] = r1s[0:32, hc, 0:32]

                    for il in range(ips):
                        ic = s * ips + il
                        isl = slice(il * 128, (il + 1) * 128)
                        # PE pre-observes relu of this slab + sco-copy(i-1)
                        pe_last = absorb_r(
                            r1s[0:32, NH - 1, il * 128:il * 128 + 32])
                        if not first_chunk[0]:
                            pe_last = order(pe_last, absorb_r(echo[:]))
                        pss = ps_s.tile([128, L], dt.float32, tag="pss")
                        # first-writer corner absorber takes the slot-recycle
                        pe_last = order(pe_last, nc.tensor.transpose(
                            pss[0:32, 0:32].bitcast(dt.float32r), identr[:],
                            identr[:]))
                        if use_mask_starter:
                            for jc in range(NJ):
                                pe_last = order(pe_last, nc.tensor.matmul(
                                    pss[:, jc * JW:(jc + 1) * JW], ones_bf[:],
                                    t_m[:, jc * JW:(jc + 1) * JW],
                                    start=True, stop=False,
                                    skip_group_check=True))
                                last_starter[0] = pe_last
                        for hc in range(NH):
                            for jc in range(NJ):
                                pe_last = order(pe_last, nc.tensor.matmul(
                                    pss[:, jc * JW:(jc + 1) * JW],
                                    r1s[:, hc, isl],
                                    r2T[:, hc, jc * JW:(jc + 1) * JW],
                                    start=(not use_mask_starter and hc == 0),
                                    stop=(hc == NH - 1),
                                    skip_group_check=True))
                        # ACT probes: observe own tail (te) then PE (pss)
                        act_last = None
                        if not first_chunk[0]:
                            act_last = nc.scalar.copy(act_ab[0:1, 0:1],
                                                      te[0:1, 0:1])
                        a = nc.scalar.copy(act_ab[0:1, 1:2], pss[0:1, 0:1])
                        act_last = order(act_last, a) if act_last else a
                        act_last = order(act_last,
                                         nc.scalar.copy(sco[:], pss[:]))
                        # fp32r echo of the copy tick for the next chunk's PE
                        act_last = order(act_last,
                                         nc.scalar.copy(echo[:],
                                                        sco[0:32, 0:32]))
                        tneg = work2p.tile([128, 1], dt.float32, tag="tneg")
                        dve_last = nc.vector.tensor_reduce(
                            tneg[:], sco[:], axis=mybir.AxisListType.X,
                            op=mybir.AluOpType.max, negate=True)
                        tnega = work2p.tile([128, 1], dt.float32, tag="tnega")
                        act_last = order(act_last,
                                         nc.scalar.copy(tnega[:], tneg[:]))
                        tz = work2p.tile([128, 1], dt.float32, tag="tz")
                        act_last = order(act_last, nc.scalar.activation(
                            te[:], sco[:], mybir.ActivationFunctionType.Exp,
                            bias=tnega[:], scale=1.0, accum_out=tz[:]))
                        # transpose E
                        for jc in range(NJ3):
                            pst = ps_sm.tile([128, 128], dt.bfloat16, tag="psm")
                            pe_last = order(pe_last, nc.tensor.transpose(
                                pst[:], te[:, jc * 128:(jc + 1) * 128],
                                identbf[:]))
                            dve_last = order(dve_last, nc.vector.tensor_copy(
                                tET[:, jc, :], pst[:]))
                        # DVE probe1: observe own tail (last ET copy)
                        dve_last = order(dve_last, nc.vector.tensor_copy(
                            dve_ab[0:1, 0:1], tET[0:1, NJ3 - 1, 0:1]))
                        tzi = work2p.tile([128, 1], dt.float32, tag="tzi")
                        dve_last = order(dve_last,
                                         nc.vector.reciprocal(tzi[:], tz[:]))
                        # bmm3 in two d-halves (1 psum bank each)
                        for dh in range(2):
                            pso = ps_sm.tile([128, 512], dt.float32, tag="psm")
                            dsl = slice(dh * 512, (dh + 1) * 512)
                            for jc in range(NJ3):
                                pe_last = order(pe_last, nc.tensor.matmul(
                                    pso[:], tET[:, jc, :],
                                    t_x2n[:, jc, dsl],
                                    start=(jc == 0), stop=(jc == NJ3 - 1)))
                                last_bmm3_mm[0] = pe_last
                            tout = outp.tile([128, 512], dt.float32,
                                             tag="tout")
                            # DVE probe2 (PE), probe3 (output-DMA WAR)
                            dve_last = order(dve_last, nc.vector.tensor_copy(
                                dve_ab[0:1, 1:2], pso[0:1, 0:1]))
                            dve_last = order(dve_last,
                                             nc.vector.memset(tout[0:1, 0:1],
                                                              0.0))
                            dve_last = order(dve_last,
                                             nc.vector.tensor_scalar_mul(
                                                 tout[:], pso[:], tzi[:]))
                            if prev_out[0] is not None:
                                sp_absorb(prev_out[0])
                            nc.sync.dma_start(
                                out[b, ic * 128:(ic + 1) * 128, dsl], tout[:])
                            prev_out[0] = out[b, ic * 128:ic * 128 + 1, dsl][:, 0:2]
                        first_chunk[0] = False
    return nc


def _prepare_inputs(x1, x2, x2_mask, W, diagonal):
    import ml_dtypes
    x1 = np.ascontiguousarray(x1, dtype=np.float32)
    x2 = np.ascontiguousarray(x2, dtype=np.float32)
    W = np.ascontiguousarray(W, dtype=np.float32)
    diagonal = np.asarray(diagonal, dtype=np.float32)
    mask = np.asarray(x2_mask).astype(np.float32)

    assert np.all(diagonal > 0), "kernel fast path requires diagonal > 0"
    WT = tf32_round(W.T.copy())
    shared_w = bool(np.all(diagonal == 1.0))
    if shared_w:
        W1T = WT
    else:
        W1T = tf32_round((W * diagonal[:, None]).T.copy())

    x1T = tf32_round(np.ascontiguousarray(x1.transpose(0, 2, 1)))
    x2T = tf32_round(np.ascontiguousarray(x2.transpose(0, 2, 1)))
    x2nb = x2.astype(ml_dtypes.bfloat16)
    mrow = ((1.0 - mask) * NEG)[:, None, :].astype(ml_dtypes.bfloat16)

    global _PROG
    if _PROG is None:
        _PROG = _build_program(separate_w1=not shared_w,
                               sw=SW if shared_w else 256)
    in_maps = []
    for c in range(NCORES):
        bs = slice(c * B_LOC, (c + 1) * B_LOC)
        in_maps.append({
            "x1T": x1T[bs],
            "x2T": x2T[bs],
            "WT": WT,
            "W1T": W1T,
            "x2n": x2nb[bs],
            "mrow": mrow[bs],
        })
    return in_maps


def _get_program():
    global _PROG
    if _PROG is None:
        _PROG = _build_program()
    return _PROG


def run(inputs, trace=False):
    """Run and return (output, BassKernelResults)."""
    from concourse.bass_utils import run_bass_kernel_spmd
    nc = _get_program()
    in_maps = _prepare_inputs(**inputs)
    res = run_bass_kernel_spmd(nc, in_maps, core_ids=list(range(NCORES)),
                               trace=trace)
    outs = [res.results[c]["out"] for c in range(NCORES)]
    full = np.concatenate(outs, axis=0).astype(np.float32)
    return full, res


# ---------------------------------------------------------------------------
# Shipping path: data-parallel jax/XLA over the 8 NeuronCores via shard_map.
# (The Bass path above compiles to IR that the current walrus rejects due to
# its 1-sync-wait-per-instruction limit on DMA queue gating; see notes.)
_JFN = None


def _jax_kernel():
    global _JFN
    if _JFN is not None:
        return _JFN
    import jax
    import jax.numpy as jnp
    from jax.sharding import Mesh, PartitionSpec as P
    from jax.experimental.shard_map import shard_map

    devices = jax.devices()[:NCORES]
    mesh = Mesh(np.asarray(devices), ("b",))

    def body(x1, x2, m, W, diag):
        r1 = jax.nn.relu(jnp.einsum("bld,hd->blh", x1, W)) * diag
        r2 = jax.nn.relu(jnp.einsum("bld,hd->blh", x2, W))
        s = jnp.einsum("bih,bjh->bij", r1, r2)
        s = s + m[:, None, :]
        a = jax.nn.softmax(s, axis=-1)
        return jnp.einsum("bij,bjd->bid", a, x2)

    fn = jax.jit(shard_map(
        body, mesh=mesh,
        in_specs=(P("b"), P("b"), P("b"), P(), P()),
        out_specs=P("b"), check_rep=False))
    _JFN = fn
    return fn


def kernel(**inputs) -> np.ndarray:
    import jax
    x1 = np.ascontiguousarray(inputs["x1"], dtype=np.float32)
    x2 = np.ascontiguousarray(inputs["x2"], dtype=np.float32)
    W = np.ascontiguousarray(inputs["W"], dtype=np.float32)
    diag = np.asarray(inputs["diagonal"], dtype=np.float32)
    m = ((1.0 - np.asarray(inputs["x2_mask"]).astype(np.float32)) * NEG)
    fn = _jax_kernel()
    out = fn(x1, x2, m.astype(np.float32), W, diag)
    return np.asarray(jax.device_get(out)).astype(np.float32)



# revision 4
# speedup vs baseline: 3.8920x; 3.8920x over previous
"""DiagonalAttention Trainium2 kernel (Bass/Tile), data-parallel over batch on 8 cores.

Reference computation (per batch b):
    r1 = relu(x1 @ W.T) * diagonal          [L, H]
    r2 = relu(x2 @ W.T)                     [L, H]
    s  = r1 @ r2.T + (1-mask)*NEG           [L, L]
    out = softmax(s, -1) @ x2               [L, D]

Per core (2 batches/core):
  - host: transpose x1/x2 to [D, L], tf32-round them + W (fp32r matmuls run at
    1 cyc/row on the PE), cast x2 to bf16 for the output matmul.
  - proj (fp32r): rT[h, l] accumulated over d-chunks, relu on ScalarE -> fp32r.
    r2T for the whole batch is resident; r1 is projected slab-by-slab.
  - scores (fp32r): per 128-row i-chunk, 4 PSUM region tiles [128,512], each a
    clean 8-matmul accumulation group; ScalarE copies each region to SBUF as it
    completes (overlaps the PE on the next region), DVE takes region maxes.
  - softmax: DVE final max (negated), ScalarE exp(s-max) -> bf16 E with fused
    row-sum (accum_out), DVE reciprocal.
  - output: E transposed 128x128 on the PE (bf16), bmm3 = ET.T @ x2_bf16
    accumulated in PSUM, scaled by 1/z on DVE during PSUM->SBUF copy, DMA out.
  - 1-chunk software pipeline: chunk i's transpose+bmm3 are emitted after chunk
    i+1's score matmuls so the PE never waits on the softmax chain.

All synchronization is left to the Tile scheduler (no manual deps).
"""
import numpy as np

B, L, D, H = 16, 2048, 1024, 1024
NCORES = 8
B_LOC = B // NCORES
NEG = -10000.0

ND = D // 128    # d chunks
NH = H // 128    # h chunks
SW = 256         # proj slab width (moving-dim of fp32r matmuls)
NS = L // SW     # slabs per batch
IPS = SW // 128  # i-chunks per slab
JW = 512         # scores region width
NJ = L // JW     # score regions per row block
NJ3 = L // 128   # j chunks in bmm3 (stationary ET tiles)

_PROGS = {}


def tf32_round(x):
    xi = np.ascontiguousarray(x, dtype=np.float32).view(np.uint32)
    return ((xi + 0x1000) & 0xFFFFE000).view(np.float32)


def _build_program(use_mask_starter=False, separate_w1=False, rep=1):
    import concourse.bass as bass
    import concourse.tile as tile
    from concourse import mybir
    from concourse.masks import make_identity

    dt = mybir.dt
    nc = bass.Bass("TRN2", target_bir_lowering=False, debug=False)

    x1T = nc.dram_tensor("x1T", [B_LOC, D, L], dt.float32r, kind="ExternalInput").ap()
    x2T = nc.dram_tensor("x2T", [B_LOC, D, L], dt.float32r, kind="ExternalInput").ap()
    WT = nc.dram_tensor("WT", [D, H], dt.float32r, kind="ExternalInput").ap()
    if separate_w1:
        W1T = nc.dram_tensor("W1T", [D, H], dt.float32r, kind="ExternalInput").ap()
    x2n = nc.dram_tensor("x2n", [B_LOC, L, D], dt.bfloat16, kind="ExternalInput").ap()
    if use_mask_starter:
        mrow = nc.dram_tensor("mrow", [B_LOC, 1, L], dt.bfloat16,
                              kind="ExternalInput").ap()
    out = nc.dram_tensor("out", [B_LOC, L, D], dt.float32, kind="ExternalOutput").ap()

    with tile.TileContext(nc) as tc:
        with (
            tc.tile_pool(name="const", bufs=1) as cpool,
            tc.tile_pool(name="wts", bufs=1) as wpool,
            tc.tile_pool(name="big", bufs=1) as bigp,
            tc.tile_pool(name="slab", bufs=2) as slabp,
            tc.tile_pool(name="r1sl", bufs=2) as r1p,
            tc.tile_pool(name="scop", bufs=1) as scop,
            tc.tile_pool(name="tep", bufs=2) as tep,
            tc.tile_pool(name="tetp", bufs=2) as tetp,
            tc.tile_pool(name="stat", bufs=2) as statp,
            tc.tile_pool(name="outp", bufs=3) as outp,
            tc.tile_pool(name="ps_p", bufs=2, space="PSUM") as ps_p,
            tc.tile_pool(name="ps_s", bufs=4, space="PSUM") as ps_s,
            tc.tile_pool(name="ps_sm", bufs=2, space="PSUM") as ps_sm,
        ):
            identbf = cpool.tile([128, 128], dt.bfloat16, tag="identbf")
            make_identity(nc, identbf[:])
            if use_mask_starter:
                ones_bf = cpool.tile([1, 128], dt.bfloat16, tag="ones_bf")
                nc.vector.memset(ones_bf[:], 1.0)

            # resident weights: wt[p, dc, h] holds W.T row d = dc*128 + p
            wt = wpool.tile([128, ND, H], dt.float32r, tag="wt")
            for dc in range(ND):
                nc.sync.dma_start(wt[:, dc, :],
                                  WT.rearrange("(c p) h -> p c h", p=128)[:, dc, :])
            if separate_w1:
                w1t = wpool.tile([128, ND, H], dt.float32r, tag="w1t")
                for dc in range(ND):
                    nc.sync.dma_start(
                        w1t[:, dc, :],
                        W1T.rearrange("(c p) h -> p c h", p=128)[:, dc, :])
            else:
                w1t = wt

            # per-batch resident tensors
            r2T = bigp.tile([128, NH, L], dt.float32r, tag="r2T")
            t_x2n = bigp.tile([128, NJ3, D], dt.bfloat16, tag="t_x2n")
            if use_mask_starter:
                t_m = bigp.tile([1, L], dt.bfloat16, tag="t_m")

            def proj_slab(src, wtile, dst_slices, s):
                """dst[:, hc, s*SW:(s+1)*SW] = relu(W @ src_slab) for all hc."""
                xs = slabp.tile([128, ND, SW], dt.float32r, tag="xs")
                for dc in range(ND):
                    nc.sync.dma_start(
                        xs[:, dc, :],
                        src.rearrange("(c p) l -> p c l", p=128)[
                            :, dc, s * SW:(s + 1) * SW])
                for hc in range(NH):
                    psp = ps_p.tile([128, SW], dt.float32, tag="psp")
                    for dc in range(ND):
                        nc.tensor.matmul(
                            psp[:], wtile[:, dc, hc * 128:(hc + 1) * 128],
                            xs[:, dc, :], start=(dc == 0), stop=(dc == ND - 1))
                    nc.scalar.activation(dst_slices(hc, s), psp[:],
                                         mybir.ActivationFunctionType.Relu)

            pend = [None]

            def flush():
                if pend[0] is None:
                    return
                b, ic, te, tzi = pend[0]
                pend[0] = None
                tET = tetp.tile([128, NJ3, 128], dt.bfloat16, tag="tET")
                for jc in range(NJ3):
                    pst = ps_sm.tile([128, 128], dt.bfloat16, tag="psm")
                    nc.tensor.transpose(pst[:], te[:, jc * 128:(jc + 1) * 128],
                                        identbf[:])
                    nc.any.tensor_copy(tET[:, jc, :], pst[:])
                for dh in range(2):
                    pso = ps_sm.tile([128, 512], dt.float32, tag="psm")
                    dsl = slice(dh * 512, (dh + 1) * 512)
                    for jc in range(NJ3):
                        nc.tensor.matmul(pso[:], tET[:, jc, :],
                                         t_x2n[:, jc, dsl],
                                         start=(jc == 0), stop=(jc == NJ3 - 1))
                    tout = outp.tile([128, 512], dt.float32, tag="tout")
                    nc.vector.tensor_scalar_mul(tout[:], pso[:], tzi[:])
                    nc.sync.dma_start(out[b, ic * 128:(ic + 1) * 128, dsl],
                                      tout[:])

            for b in [bb for _ in range(rep) for bb in range(B_LOC)]:
                # flush last chunk of the previous batch before t_x2n reload
                flush()
                for jc in range(NJ3):
                    nc.sync.dma_start(
                        t_x2n[:, jc, :],
                        x2n[b].rearrange("(c p) d -> p c d", p=128)[:, jc, :])
                if use_mask_starter:
                    nc.sync.dma_start(t_m[:], mrow[b])

                # proj2: r2T = relu(W @ x2T), whole batch resident
                for s in range(NS):
                    proj_slab(x2T[b], wt,
                              lambda hc, s: r2T[:, hc, s * SW:(s + 1) * SW], s)

                # proj1 + attention, slab by slab
                for s in range(NS):
                    r1s = r1p.tile([128, NH, SW], dt.float32r, tag="r1s")
                    proj_slab(x1T[b], w1t, lambda hc, _s: r1s[:, hc, :], s)

                    for il in range(IPS):
                        ic = s * IPS + il
                        isl = slice(il * 128, (il + 1) * 128)
                        sco = scop.tile([128, L], dt.float32, tag="sco")
                        tneg4 = statp.tile([128, NJ], dt.float32, tag="tneg4")
                        for jc in range(NJ):
                            jsl = slice(jc * JW, (jc + 1) * JW)
                            pss = ps_s.tile([128, JW], dt.float32, tag="pss")
                            if use_mask_starter:
                                nc.tensor.matmul(pss[:], ones_bf[:],
                                                 t_m[:, jsl],
                                                 start=True, stop=False)
                            for hc in range(NH):
                                nc.tensor.matmul(
                                    pss[:], r1s[:, hc, isl],
                                    r2T[:, hc, jsl],
                                    start=(hc == 0 and not use_mask_starter),
                                    stop=(hc == NH - 1))
                            nc.scalar.copy(sco[:, jsl], pss[:])
                            nc.vector.tensor_reduce(
                                tneg4[:, jc:jc + 1], sco[:, jsl],
                                axis=mybir.AxisListType.X,
                                op=mybir.AluOpType.max)
                        tnega = statp.tile([128, 1], dt.float32, tag="tnega")
                        nc.vector.tensor_reduce(
                            tnega[:], tneg4[:], axis=mybir.AxisListType.X,
                            op=mybir.AluOpType.max, negate=True)
                        te = tep.tile([128, L], dt.bfloat16, tag="te")
                        tz = statp.tile([128, 1], dt.float32, tag="tz")
                        nc.scalar.activation(
                            te[:], sco[:], mybir.ActivationFunctionType.Exp,
                            bias=tnega[:], scale=1.0, accum_out=tz[:])
                        tzi = statp.tile([128, 1], dt.float32, tag="tzi")
                        nc.vector.reciprocal(tzi[:], tz[:])
                        # emit previous chunk's transpose+bmm3 now so the PE
                        # has dense work while this chunk's softmax runs
                        flush()
                        pend[0] = (b, ic, te, tzi)
            flush()
    return nc


def _fix_multi_waits(bj: bytes) -> bytes:
    """Split multi-semaphore waits into single-wait NoOp carriers.

    The installed walrus rejects any engine/DMA instruction carrying more
    than one sync-wait command, but the Tile scheduler emits up to three.
    Each engine's NX sequencer executes its stream in order, so hoisting
    the surplus waits onto NoOp instructions inserted immediately before
    the original preserves exactly the same gating.
    """
    import json
    m = json.loads(bj)
    SKIP = {"EventSemaphore", "Call", "UnconditionalBranch"}
    for fn in m["functions"]:
        for bl in fn["blocks"]:
            out = []
            for inst in bl["instructions"]:
                si = inst.get("sync_info")
                waits = (si or {}).get("on_wait") or []
                if len(waits) > 1 and inst.get("opcode") not in SKIP:
                    for k, w in enumerate(waits[:-1]):
                        out.append({
                            "debug": inst.get("debug", 0),
                            "engine": inst["engine"],
                            "ins": [], "outs": [],
                            "name": f"{inst['name']}w{k}",
                            "opcode": "NoOp",
                            "sync_info": {"on_update": [], "on_wait": [w]},
                        })
                    si = dict(si)
                    si["on_wait"] = [waits[-1]]
                    inst = dict(inst)
                    inst["sync_info"] = si
                out.append(inst)
            bl["instructions"] = out
    return json.dumps(m).encode()


def _get_program(use_mask_starter=False, separate_w1=False, rep=1):
    key = (use_mask_starter, separate_w1, rep)
    if key not in _PROGS:
        nc = _build_program(use_mask_starter, separate_w1, rep)
        fixed = _fix_multi_waits(nc.to_json_bytes())
        nc.to_json_bytes = lambda: fixed
        _PROGS[key] = nc
    return _PROGS[key]


def _prepare_inputs(x1, x2, x2_mask, W, diagonal, rep=1):
    import ml_dtypes
    x1 = np.ascontiguousarray(x1, dtype=np.float32)
    x2 = np.ascontiguousarray(x2, dtype=np.float32)
    W = np.ascontiguousarray(W, dtype=np.float32)
    diag = np.asarray(diagonal, dtype=np.float32)
    mask = np.asarray(x2_mask).astype(np.float32)

    shared_w = bool(np.all(diag == 1.0))
    if not shared_w:
        # fold diagonal into W1 (requires diag >= 0 so relu commutes)
        assert np.all(diag >= 0), "kernel fast path requires diagonal >= 0"
    trivial_mask = bool(np.all(mask == 1.0))

    WT = tf32_round(W.T.copy())
    W1T = WT if shared_w else tf32_round((W * diag[:, None]).T.copy())
    x1T = tf32_round(np.ascontiguousarray(x1.transpose(0, 2, 1)))
    x2T = tf32_round(np.ascontiguousarray(x2.transpose(0, 2, 1)))
    x2nb = x2.astype(ml_dtypes.bfloat16)
    mrow = ((1.0 - mask) * NEG)[:, None, :].astype(ml_dtypes.bfloat16)

    nc = _get_program(use_mask_starter=not trivial_mask,
                      separate_w1=not shared_w, rep=rep)
    in_maps = []
    for c in range(NCORES):
        bs = slice(c * B_LOC, (c + 1) * B_LOC)
        m = {"x1T": x1T[bs], "x2T": x2T[bs], "WT": WT, "x2n": x2nb[bs]}
        if not shared_w:
            m["W1T"] = W1T
        if not trivial_mask:
            m["mrow"] = mrow[bs]
        in_maps.append(m)
    return nc, in_maps


def run(inputs, trace=False, rep=1, **kw):
    """Run and return (output, BassKernelResults)."""
    from concourse.bass_utils import run_bass_kernel_spmd
    nc, in_maps = _prepare_inputs(**inputs, rep=rep)
    res = run_bass_kernel_spmd(nc, in_maps, core_ids=list(range(NCORES)),
                               trace=trace, **kw)
    outs = [res.results[c]["out"] for c in range(NCORES)]
    full = np.concatenate(outs, axis=0).astype(np.float32)
    return full, res


_EXECS = {}


def _get_executor(nc):
    """Build (once per program) a jitted PJRT callable for the Bass module.

    Replicates bass2jax.run_bass_via_pjrt's multi-core branch, but caches the
    jitted function so repeated calls skip XLA retracing/recompilation.
    """
    if id(nc) in _EXECS:
        return _EXECS[id(nc)]
    import jax
    from jax.sharding import Mesh, PartitionSpec, NamedSharding
    from jax.experimental.shard_map import shard_map
    from concourse import bass2jax, mybir

    bass2jax.install_neuronx_cc_hook()

    partition_name = (nc.partition_id_tensor.name
                      if nc.partition_id_tensor else None)
    in_names, out_names, out_avals, zero_outs = [], [], [], []
    for alloc in nc.m.functions[0].allocations:
        if not isinstance(alloc, mybir.MemoryLocationSet):
            continue
        name = alloc.memorylocations[0].name
        if alloc.kind == "ExternalInput":
            if name != partition_name:
                in_names.append(name)
        elif alloc.kind == "ExternalOutput":
            out_names.append(name)
            shape = tuple(alloc.tensor_shape)
            dtype = mybir.dt.np(alloc.dtype)
            out_avals.append(jax.core.ShapedArray(shape, dtype))
            zero_outs.append(np.zeros(shape, dtype))
    n_params = len(in_names)
    n_outs = len(out_avals)
    all_names = in_names + out_names
    if partition_name is not None:
        all_names = all_names + [partition_name]
    donate = tuple(range(n_params, n_params + n_outs))

    def _body(*args):
        operands = list(args)
        if partition_name is not None:
            operands.append(bass2jax.partition_id_tensor())
        outs = bass2jax._bass_exec_p.bind(
            *operands,
            out_avals=tuple(out_avals),
            in_names=tuple(all_names),
            out_names=tuple(out_names),
            lowering_input_output_aliases=(),
            sim_require_finite=True,
            sim_require_nnan=True,
            nc=nc,
        )
        return tuple(outs)

    devices = jax.devices()[:NCORES]
    mesh = Mesh(np.asarray(devices), ("core",))
    sharded = jax.jit(
        shard_map(_body, mesh=mesh,
                  in_specs=(PartitionSpec("core"),) * (n_params + n_outs),
                  out_specs=(PartitionSpec("core"),) * n_outs,
                  check_rep=False),
        donate_argnums=donate, keep_unused=True)

    shb = NamedSharding(mesh, PartitionSpec("core"))
    ex = {"sharded": sharded, "in_names": in_names, "out_names": out_names,
          "out_avals": out_avals, "zero_outs": zero_outs, "shb": shb}
    _EXECS[id(nc)] = ex
    return ex


def _execute(nc, in_maps):
    """One kernel execution through the cached PJRT executable."""
    import jax
    ex = _get_executor(nc)
    concat_in = [
        jax.device_put(
            np.concatenate([np.asarray(in_maps[c][nm])
                            for c in range(NCORES)], axis=0), ex["shb"])
        for nm in ex["in_names"]
    ]
    zo = [jax.device_put(
        np.zeros((NCORES * z.shape[0], *z.shape[1:]), z.dtype), ex["shb"])
        for z in ex["zero_outs"]]
    outs = ex["sharded"](*concat_in, *zo)
    jax.block_until_ready(outs)
    return {nm: np.asarray(outs[i]) for i, nm in enumerate(ex["out_names"])}


def make_bench(inputs, rep=1):
    """Build a zero-transfer callable for repeated timed executions.

    Inputs stay device-resident across calls; output buffers ping-pong
    through the donated slots, so each call is one dispatch + one device
    execution of the whole kernel (x rep).
    """
    import jax
    nc, in_maps = _prepare_inputs(**inputs, rep=rep)
    ex = _get_executor(nc)
    concat_in = [
        jax.device_put(
            np.concatenate([np.asarray(in_maps[c][nm])
                            for c in range(NCORES)], axis=0), ex["shb"])
        for nm in ex["in_names"]
    ]
    state = {"outs": None}

    def call():
        if state["outs"] is None:
            zo = [jax.device_put(
                np.zeros((NCORES * z.shape[0], *z.shape[1:]), z.dtype),
                ex["shb"]) for z in ex["zero_outs"]]
        else:
            zo = list(state["outs"])
        outs = ex["sharded"](*concat_in, *zo)
        jax.block_until_ready(outs)
        state["outs"] = outs
        return outs

    return call


def _kernel_bass(**inputs) -> np.ndarray:
    nc, in_maps = _prepare_inputs(**inputs)
    outs = _execute(nc, in_maps)
    return np.ascontiguousarray(
        outs["out"].reshape(NCORES * B_LOC, L, D).astype(np.float32))


def _kernel_jax(**inputs) -> np.ndarray:
    """Fallback: data-parallel XLA over the 8 NeuronCores via shard_map."""
    import jax
    import jax.numpy as jnp
    from jax.sharding import Mesh, PartitionSpec as P
    from jax.experimental.shard_map import shard_map

    x1 = np.ascontiguousarray(inputs["x1"], dtype=np.float32)
    x2 = np.ascontiguousarray(inputs["x2"], dtype=np.float32)
    W = np.ascontiguousarray(inputs["W"], dtype=np.float32)
    diag = np.asarray(inputs["diagonal"], dtype=np.float32)
    m = ((1.0 - np.asarray(inputs["x2_mask"]).astype(np.float32)) * NEG)

    devices = jax.devices()[:NCORES]
    mesh = Mesh(np.asarray(devices), ("b",))

    def body(x1, x2, m, W, diag):
        r1 = jax.nn.relu(jnp.einsum("bld,hd->blh", x1, W)) * diag
        r2 = jax.nn.relu(jnp.einsum("bld,hd->blh", x2, W))
        s = jnp.einsum("bih,bjh->bij", r1, r2) + m[:, None, :]
        a = jax.nn.softmax(s, axis=-1)
        return jnp.einsum("bij,bjd->bid", a, x2)

    fn = jax.jit(shard_map(
        body, mesh=mesh,
        in_specs=(P("b"), P("b"), P("b"), P(), P()),
        out_specs=P("b"), check_rep=False))
    out = fn(x1, x2, m.astype(np.float32), W, diag)
    return np.asarray(jax.device_get(out)).astype(np.float32)


def kernel(**inputs) -> np.ndarray:
    try:
        return _kernel_bass(**inputs)
    except Exception:
        import traceback
        traceback.print_exc()
        return _kernel_jax(**inputs)


# revision 5
# speedup vs baseline: 4.0063x; 1.0294x over previous
"""DiagonalAttention Trainium2 kernel (Bass/Tile), data-parallel over batch on 8 cores.

Reference computation (per batch b):
    r1 = relu(x1 @ W.T) * diagonal          [L, H]
    r2 = relu(x2 @ W.T)                     [L, H]
    s  = r1 @ r2.T + (1-mask)*NEG           [L, L]
    out = softmax(s, -1) @ x2               [L, D]

Per core (2 batches/core):
  - host: transpose x1/x2 to [D, L], tf32-round them + W (fp32r matmuls run at
    1 cyc/row on the PE), cast x2 to bf16 for the output matmul.
  - proj (fp32r): rT[h, l] accumulated over d-chunks, relu on ScalarE -> fp32r.
    r2T for the whole batch is resident; r1 is projected slab-by-slab.
  - scores (fp32r): per 128-row i-chunk, 4 PSUM region tiles [128,512], each a
    clean 8-matmul accumulation group; ScalarE copies each region to SBUF as it
    completes (overlaps the PE on the next region), DVE takes region maxes.
  - softmax: DVE final max (negated), ScalarE exp(s-max) -> bf16 E with fused
    row-sum (accum_out), DVE reciprocal.
  - output: E transposed 128x128 on the PE (bf16), bmm3 = ET.T @ x2_bf16
    accumulated in PSUM, scaled by 1/z on DVE during PSUM->SBUF copy, DMA out.
  - 1-chunk software pipeline: chunk i's transpose+bmm3 are emitted after chunk
    i+1's score matmuls so the PE never waits on the softmax chain.

All synchronization is left to the Tile scheduler (no manual deps).
"""
import numpy as np

B, L, D, H = 16, 2048, 1024, 1024
NCORES = 8
B_LOC = B // NCORES
NEG = -10000.0

ND = D // 128    # d chunks
NH = H // 128    # h chunks
SW = 256         # proj slab width (moving-dim of fp32r matmuls)
NS = L // SW     # slabs per batch
IPS = SW // 128  # i-chunks per slab
JW = 512         # scores region width
NJ = L // JW     # score regions per row block
NJ3 = L // 128   # j chunks in bmm3 (stationary ET tiles)

_PROGS = {}


def tf32_round(x):
    xi = np.ascontiguousarray(x, dtype=np.float32).view(np.uint32)
    return ((xi + 0x1000) & 0xFFFFE000).view(np.float32)


def _build_program(use_mask_starter=False, separate_w1=False, rep=1):
    import concourse.bass as bass
    import concourse.tile as tile
    from concourse import mybir
    from concourse.masks import make_identity

    dt = mybir.dt
    nc = bass.Bass("TRN2", target_bir_lowering=False, debug=False)

    x1T = nc.dram_tensor("x1T", [B_LOC, D, L], dt.float32r, kind="ExternalInput").ap()
    x2T = nc.dram_tensor("x2T", [B_LOC, D, L], dt.float32r, kind="ExternalInput").ap()
    WT = nc.dram_tensor("WT", [D, H], dt.float32r, kind="ExternalInput").ap()
    if separate_w1:
        W1T = nc.dram_tensor("W1T", [D, H], dt.float32r, kind="ExternalInput").ap()
    x2n = nc.dram_tensor("x2n", [B_LOC, L, D], dt.bfloat16, kind="ExternalInput").ap()
    if use_mask_starter:
        mrow = nc.dram_tensor("mrow", [B_LOC, 1, L], dt.bfloat16,
                              kind="ExternalInput").ap()
    out = nc.dram_tensor("out", [B_LOC, L, D], dt.float32, kind="ExternalOutput").ap()

    with tile.TileContext(nc) as tc:
        with (
            tc.tile_pool(name="const", bufs=1) as cpool,
            tc.tile_pool(name="wts", bufs=1) as wpool,
            tc.tile_pool(name="big", bufs=1) as bigp,
            tc.tile_pool(name="slab", bufs=2) as slabp,
            tc.tile_pool(name="r1sl", bufs=2) as r1p,
            tc.tile_pool(name="scop", bufs=1) as scop,
            tc.tile_pool(name="tep", bufs=2) as tep,
            tc.tile_pool(name="tetp", bufs=2) as tetp,
            tc.tile_pool(name="stat", bufs=2) as statp,
            tc.tile_pool(name="outp", bufs=3) as outp,
            tc.tile_pool(name="ps_p", bufs=2, space="PSUM") as ps_p,
            tc.tile_pool(name="ps_s", bufs=2, space="PSUM") as ps_s,
            tc.tile_pool(name="ps_sm", bufs=4, space="PSUM") as ps_sm,
        ):
            identbf = cpool.tile([128, 128], dt.bfloat16, tag="identbf")
            make_identity(nc, identbf[:])
            if use_mask_starter:
                ones_bf = cpool.tile([1, 128], dt.bfloat16, tag="ones_bf")
                nc.vector.memset(ones_bf[:], 1.0)

            # resident weights: wt[p, dc, h] holds W.T row d = dc*128 + p
            wt = wpool.tile([128, ND, H], dt.float32r, tag="wt")
            for dc in range(ND):
                nc.sync.dma_start(wt[:, dc, :],
                                  WT.rearrange("(c p) h -> p c h", p=128)[:, dc, :])
            if separate_w1:
                w1t = wpool.tile([128, ND, H], dt.float32r, tag="w1t")
                for dc in range(ND):
                    nc.sync.dma_start(
                        w1t[:, dc, :],
                        W1T.rearrange("(c p) h -> p c h", p=128)[:, dc, :])
            else:
                w1t = wt

            # per-batch resident tensors
            r2T = bigp.tile([128, NH, L], dt.float32r, tag="r2T")
            t_x2n = bigp.tile([128, NJ3, D], dt.bfloat16, tag="t_x2n")
            if use_mask_starter:
                t_m = bigp.tile([1, L], dt.bfloat16, tag="t_m")

            def proj_slab(src, wtile, dst_slices, s):
                """dst[:, hc, s*SW:(s+1)*SW] = relu(W @ src_slab) for all hc."""
                xs = slabp.tile([128, ND, SW], dt.float32r, tag="xs")
                for dc in range(ND):
                    nc.sync.dma_start(
                        xs[:, dc, :],
                        src.rearrange("(c p) l -> p c l", p=128)[
                            :, dc, s * SW:(s + 1) * SW])
                for hc in range(NH):
                    psp = ps_p.tile([128, SW], dt.float32, tag="psp")
                    for dc in range(ND):
                        nc.tensor.matmul(
                            psp[:], wtile[:, dc, hc * 128:(hc + 1) * 128],
                            xs[:, dc, :], start=(dc == 0), stop=(dc == ND - 1))
                    nc.scalar.activation(dst_slices(hc, s), psp[:],
                                         mybir.ActivationFunctionType.Relu)

            pend = [None]

            def flush():
                if pend[0] is None:
                    return
                b, ic, te, tzi = pend[0]
                pend[0] = None
                tET = tetp.tile([128, NJ3, 128], dt.bfloat16, tag="tET")
                for jc in range(NJ3):
                    pst = ps_sm.tile([128, 128], dt.bfloat16, tag="psm")
                    nc.tensor.transpose(pst[:], te[:, jc * 128:(jc + 1) * 128],
                                        identbf[:])
                    nc.any.tensor_copy(tET[:, jc, :], pst[:])
                for dh in range(2):
                    pso = ps_sm.tile([128, 512], dt.float32, tag="psm")
                    dsl = slice(dh * 512, (dh + 1) * 512)
                    for jc in range(NJ3):
                        nc.tensor.matmul(pso[:], tET[:, jc, :],
                                         t_x2n[:, jc, dsl],
                                         start=(jc == 0), stop=(jc == NJ3 - 1))
                    tout = outp.tile([128, 512], dt.float32, tag="tout")
                    nc.vector.tensor_scalar_mul(tout[:], pso[:], tzi[:])
                    nc.sync.dma_start(out[b, ic * 128:(ic + 1) * 128, dsl],
                                      tout[:])

            for b in [bb for _ in range(rep) for bb in range(B_LOC)]:
                # flush last chunk of the previous batch before t_x2n reload
                flush()
                for jc in range(NJ3):
                    nc.sync.dma_start(
                        t_x2n[:, jc, :],
                        x2n[b].rearrange("(c p) d -> p c d", p=128)[:, jc, :])
                if use_mask_starter:
                    nc.sync.dma_start(t_m[:], mrow[b])

                # proj2: r2T = relu(W @ x2T), whole batch resident
                for s in range(NS):
                    proj_slab(x2T[b], wt,
                              lambda hc, s: r2T[:, hc, s * SW:(s + 1) * SW], s)

                # proj1 + attention, slab by slab
                for s in range(NS):
                    r1s = r1p.tile([128, NH, SW], dt.float32r, tag="r1s")
                    proj_slab(x1T[b], w1t, lambda hc, _s: r1s[:, hc, :], s)

                    for il in range(IPS):
                        ic = s * IPS + il
                        isl = slice(il * 128, (il + 1) * 128)
                        sco = scop.tile([128, L], dt.float32, tag="sco")
                        tneg4 = statp.tile([128, NJ], dt.float32, tag="tneg4")
                        for jc in range(NJ):
                            jsl = slice(jc * JW, (jc + 1) * JW)
                            pss = ps_s.tile([128, JW], dt.float32, tag="pss")
                            if use_mask_starter:
                                nc.tensor.matmul(pss[:], ones_bf[:],
                                                 t_m[:, jsl],
                                                 start=True, stop=False)
                            for hc in range(NH):
                                nc.tensor.matmul(
                                    pss[:], r1s[:, hc, isl],
                                    r2T[:, hc, jsl],
                                    start=(hc == 0 and not use_mask_starter),
                                    stop=(hc == NH - 1))
                            nc.scalar.copy(sco[:, jsl], pss[:])
                            nc.vector.tensor_reduce(
                                tneg4[:, jc:jc + 1], sco[:, jsl],
                                axis=mybir.AxisListType.X,
                                op=mybir.AluOpType.max)
                        tnega = statp.tile([128, 1], dt.float32, tag="tnega")
                        nc.vector.tensor_reduce(
                            tnega[:], tneg4[:], axis=mybir.AxisListType.X,
                            op=mybir.AluOpType.max, negate=True)
                        te = tep.tile([128, L], dt.bfloat16, tag="te")
                        tz = statp.tile([128, 1], dt.float32, tag="tz")
                        nc.scalar.activation(
                            te[:], sco[:], mybir.ActivationFunctionType.Exp,
                            bias=tnega[:], scale=1.0, accum_out=tz[:])
                        tzi = statp.tile([128, 1], dt.float32, tag="tzi")
                        nc.vector.reciprocal(tzi[:], tz[:])
                        # emit previous chunk's transpose+bmm3 now so the PE
                        # has dense work while this chunk's softmax runs
                        flush()
                        pend[0] = (b, ic, te, tzi)
            flush()
    return nc


def _prune_sem_incs(m: dict) -> None:
    """Drop engine-semaphore increments that no wait ever observes.

    Every engine instruction Tile emits bumps its engine's semaphore, and
    each bump costs ~26 ns of issue bandwidth on the engine (serialized
    EVT_SEM register write). Only a few hundred of those ticks are actual
    wait thresholds. For each inc-by-1 semaphore, keep exactly the incs
    whose cumulative count equals some waited threshold and renumber all
    wait values to the kept-inc rank — an exact, order-preserving rewrite.
    """
    import bisect
    # classify sems: only touch sems where every update is +1 sem-inc and
    # every wait is sem-ge-imm
    ok = {}
    thresholds = {}
    for fn in m["functions"]:
        for bl in fn["blocks"]:
            for inst in bl["instructions"]:
                si = inst.get("sync_info") or {}
                for u in si.get("on_update") or []:
                    s = u["ant_name"]
                    good = (u.get("sync_type") == "semaphore"
                            and u.get("update_mode") == "sem-inc"
                            and u.get("update_value") == 1)
                    ok[s] = ok.get(s, True) and good
                for w in si.get("on_wait") or []:
                    s = w["ant_name"]
                    if (w.get("sync_type") != "semaphore"
                            or w.get("wait_mode") != "sem-ge-imm"):
                        ok[s] = False
                    else:
                        thresholds.setdefault(s, set()).add(w["wait_value"])
    keep = {s for s, good in ok.items() if good}
    tsort = {s: sorted(thresholds.get(s, ())) for s in keep}
    cum = {s: 0 for s in keep}
    for fn in m["functions"]:
        for bl in fn["blocks"]:
            for inst in bl["instructions"]:
                si = inst.get("sync_info")
                if not si:
                    continue
                ups = si.get("on_update") or []
                new_ups = []
                for u in ups:
                    s = u["ant_name"]
                    if s not in keep:
                        new_ups.append(u)
                        continue
                    cum[s] += 1
                    if cum[s] in thresholds.get(s, ()):
                        new_ups.append(u)
                si["on_update"] = new_ups
                for w in si.get("on_wait") or []:
                    s = w["ant_name"]
                    if s in keep:
                        w["wait_value"] = bisect.bisect_right(
                            tsort[s], w["wait_value"])


def _fix_multi_waits(bj: bytes) -> bytes:
    """Split multi-semaphore waits into single-wait NoOp carriers.

    The installed walrus rejects any engine/DMA instruction carrying more
    than one sync-wait command, but the Tile scheduler emits up to three.
    Each engine's NX sequencer executes its stream in order, so hoisting
    the surplus waits onto NoOp instructions inserted immediately before
    the original preserves exactly the same gating.
    """
    import json
    m = json.loads(bj)
    _prune_sem_incs(m)
    SKIP = {"EventSemaphore", "Call", "UnconditionalBranch"}
    for fn in m["functions"]:
        for bl in fn["blocks"]:
            out = []
            for inst in bl["instructions"]:
                si = inst.get("sync_info")
                waits = (si or {}).get("on_wait") or []
                if len(waits) > 1 and inst.get("opcode") not in SKIP:
                    for k, w in enumerate(waits[:-1]):
                        out.append({
                            "debug": inst.get("debug", 0),
                            "engine": inst["engine"],
                            "ins": [], "outs": [],
                            "name": f"{inst['name']}w{k}",
                            "opcode": "NoOp",
                            "sync_info": {"on_update": [], "on_wait": [w]},
                        })
                    si = dict(si)
                    si["on_wait"] = [waits[-1]]
                    inst = dict(inst)
                    inst["sync_info"] = si
                out.append(inst)
            bl["instructions"] = out
    return json.dumps(m).encode()


def _get_program(use_mask_starter=False, separate_w1=False, rep=1):
    key = (use_mask_starter, separate_w1, rep)
    if key not in _PROGS:
        nc = _build_program(use_mask_starter, separate_w1, rep)
        fixed = _fix_multi_waits(nc.to_json_bytes())
        nc.to_json_bytes = lambda: fixed
        _PROGS[key] = nc
    return _PROGS[key]


def _prepare_inputs(x1, x2, x2_mask, W, diagonal, rep=1):
    import ml_dtypes
    x1 = np.ascontiguousarray(x1, dtype=np.float32)
    x2 = np.ascontiguousarray(x2, dtype=np.float32)
    W = np.ascontiguousarray(W, dtype=np.float32)
    diag = np.asarray(diagonal, dtype=np.float32)
    mask = np.asarray(x2_mask).astype(np.float32)

    shared_w = bool(np.all(diag == 1.0))
    if not shared_w:
        # fold diagonal into W1 (requires diag >= 0 so relu commutes)
        assert np.all(diag >= 0), "kernel fast path requires diagonal >= 0"
    trivial_mask = bool(np.all(mask == 1.0))

    WT = tf32_round(W.T.copy())
    W1T = WT if shared_w else tf32_round((W * diag[:, None]).T.copy())
    x1T = tf32_round(np.ascontiguousarray(x1.transpose(0, 2, 1)))
    x2T = tf32_round(np.ascontiguousarray(x2.transpose(0, 2, 1)))
    x2nb = x2.astype(ml_dtypes.bfloat16)
    mrow = ((1.0 - mask) * NEG)[:, None, :].astype(ml_dtypes.bfloat16)

    nc = _get_program(use_mask_starter=not trivial_mask,
                      separate_w1=not shared_w, rep=rep)
    in_maps = []
    for c in range(NCORES):
        bs = slice(c * B_LOC, (c + 1) * B_LOC)
        m = {"x1T": x1T[bs], "x2T": x2T[bs], "WT": WT, "x2n": x2nb[bs]}
        if not shared_w:
            m["W1T"] = W1T
        if not trivial_mask:
            m["mrow"] = mrow[bs]
        in_maps.append(m)
    return nc, in_maps


def run(inputs, trace=False, rep=1, **kw):
    """Run and return (output, BassKernelResults)."""
    from concourse.bass_utils import run_bass_kernel_spmd
    nc, in_maps = _prepare_inputs(**inputs, rep=rep)
    res = run_bass_kernel_spmd(nc, in_maps, core_ids=list(range(NCORES)),
                               trace=trace, **kw)
    outs = [res.results[c]["out"] for c in range(NCORES)]
    full = np.concatenate(outs, axis=0).astype(np.float32)
    return full, res


_EXECS = {}


def _get_executor(nc):
    """Build (once per program) a jitted PJRT callable for the Bass module.

    Replicates bass2jax.run_bass_via_pjrt's multi-core branch, but caches the
    jitted function so repeated calls skip XLA retracing/recompilation.
    """
    if id(nc) in _EXECS:
        return _EXECS[id(nc)]
    import jax
    from jax.sharding import Mesh, PartitionSpec, NamedSharding
    from jax.experimental.shard_map import shard_map
    from concourse import bass2jax, mybir

    bass2jax.install_neuronx_cc_hook()

    partition_name = (nc.partition_id_tensor.name
                      if nc.partition_id_tensor else None)
    in_names, out_names, out_avals, zero_outs = [], [], [], []
    for alloc in nc.m.functions[0].allocations:
        if not isinstance(alloc, mybir.MemoryLocationSet):
            continue
        name = alloc.memorylocations[0].name
        if alloc.kind == "ExternalInput":
            if name != partition_name:
                in_names.append(name)
        elif alloc.kind == "ExternalOutput":
            out_names.append(name)
            shape = tuple(alloc.tensor_shape)
            dtype = mybir.dt.np(alloc.dtype)
            out_avals.append(jax.core.ShapedArray(shape, dtype))
            zero_outs.append(np.zeros(shape, dtype))
    n_params = len(in_names)
    n_outs = len(out_avals)
    all_names = in_names + out_names
    if partition_name is not None:
        all_names = all_names + [partition_name]
    donate = tuple(range(n_params, n_params + n_outs))

    def _body(*args):
        operands = list(args)
        if partition_name is not None:
            operands.append(bass2jax.partition_id_tensor())
        outs = bass2jax._bass_exec_p.bind(
            *operands,
            out_avals=tuple(out_avals),
            in_names=tuple(all_names),
            out_names=tuple(out_names),
            lowering_input_output_aliases=(),
            sim_require_finite=True,
            sim_require_nnan=True,
            nc=nc,
        )
        return tuple(outs)

    devices = jax.devices()[:NCORES]
    mesh = Mesh(np.asarray(devices), ("core",))
    sharded = jax.jit(
        shard_map(_body, mesh=mesh,
                  in_specs=(PartitionSpec("core"),) * (n_params + n_outs),
                  out_specs=(PartitionSpec("core"),) * n_outs,
                  check_rep=False),
        donate_argnums=donate, keep_unused=True)

    shb = NamedSharding(mesh, PartitionSpec("core"))
    ex = {"sharded": sharded, "in_names": in_names, "out_names": out_names,
          "out_avals": out_avals, "zero_outs": zero_outs, "shb": shb}
    _EXECS[id(nc)] = ex
    return ex


def _execute(nc, in_maps):
    """One kernel execution through the cached PJRT executable."""
    import jax
    ex = _get_executor(nc)
    concat_in = [
        jax.device_put(
            np.concatenate([np.asarray(in_maps[c][nm])
                            for c in range(NCORES)], axis=0), ex["shb"])
        for nm in ex["in_names"]
    ]
    zo = [jax.device_put(
        np.zeros((NCORES * z.shape[0], *z.shape[1:]), z.dtype), ex["shb"])
        for z in ex["zero_outs"]]
    outs = ex["sharded"](*concat_in, *zo)
    jax.block_until_ready(outs)
    return {nm: np.asarray(outs[i]) for i, nm in enumerate(ex["out_names"])}


def make_bench(inputs, rep=1):
    """Build a zero-transfer callable for repeated timed executions.

    Inputs stay device-resident across calls; output buffers ping-pong
    through the donated slots, so each call is one dispatch + one device
    execution of the whole kernel (x rep).
    """
    import jax
    nc, in_maps = _prepare_inputs(**inputs, rep=rep)
    ex = _get_executor(nc)
    concat_in = [
        jax.device_put(
            np.concatenate([np.asarray(in_maps[c][nm])
                            for c in range(NCORES)], axis=0), ex["shb"])
        for nm in ex["in_names"]
    ]
    state = {"outs": None}

    def call():
        if state["outs"] is None:
            zo = [jax.device_put(
                np.zeros((NCORES * z.shape[0], *z.shape[1:]), z.dtype),
                ex["shb"]) for z in ex["zero_outs"]]
        else:
            zo = list(state["outs"])
        outs = ex["sharded"](*concat_in, *zo)
        jax.block_until_ready(outs)
        state["outs"] = outs
        return outs

    return call


def _kernel_bass(**inputs) -> np.ndarray:
    nc, in_maps = _prepare_inputs(**inputs)
    outs = _execute(nc, in_maps)
    return np.ascontiguousarray(
        outs["out"].reshape(NCORES * B_LOC, L, D).astype(np.float32))


def _kernel_jax(**inputs) -> np.ndarray:
    """Fallback: data-parallel XLA over the 8 NeuronCores via shard_map."""
    import jax
    import jax.numpy as jnp
    from jax.sharding import Mesh, PartitionSpec as P
    from jax.experimental.shard_map import shard_map

    x1 = np.ascontiguousarray(inputs["x1"], dtype=np.float32)
    x2 = np.ascontiguousarray(inputs["x2"], dtype=np.float32)
    W = np.ascontiguousarray(inputs["W"], dtype=np.float32)
    diag = np.asarray(inputs["diagonal"], dtype=np.float32)
    m = ((1.0 - np.asarray(inputs["x2_mask"]).astype(np.float32)) * NEG)

    devices = jax.devices()[:NCORES]
    mesh = Mesh(np.asarray(devices), ("b",))

    def body(x1, x2, m, W, diag):
        r1 = jax.nn.relu(jnp.einsum("bld,hd->blh", x1, W)) * diag
        r2 = jax.nn.relu(jnp.einsum("bld,hd->blh", x2, W))
        s = jnp.einsum("bih,bjh->bij", r1, r2) + m[:, None, :]
        a = jax.nn.softmax(s, axis=-1)
        return jnp.einsum("bij,bjd->bid", a, x2)

    fn = jax.jit(shard_map(
        body, mesh=mesh,
        in_specs=(P("b"), P("b"), P("b"), P(), P()),
        out_specs=P("b"), check_rep=False))
    out = fn(x1, x2, m.astype(np.float32), W, diag)
    return np.asarray(jax.device_get(out)).astype(np.float32)


def kernel(**inputs) -> np.ndarray:
    try:
        return _kernel_bass(**inputs)
    except Exception:
        import traceback
        traceback.print_exc()
        return _kernel_jax(**inputs)
